# revision 8
# baseline (speedup 1.0000x reference)
"""Multi-resolution 1D ROI max-pooling kernel for Trainium2 (raw Bass).

Reference computation: x[4096, 16384] f32; for each pool width p in
[1, 2, 4, 8, 16] max-pool the W dim into p equal bins (16384 % 16 == 0 so
all bins are exact), concatenate -> out[4096, 31] with column layout
[m1 | m2(2) | m4(4) | m8(8) | m16(16)].

Strategy: pure data parallel over the batch dim -- 8 cores x 512 rows.
Per core, 4 row-tiles of [128, 16384] are DMAed to SBUF (double-buffered);
one segmented reduce_max produces the 16 finest bins, then a cascade of
tiny pairwise reduces produces the coarser levels (max is hierarchical).
All 4 row-tiles' results accumulate in one small SBUF tile, flushed with a
single DMA at the end.

Raw Bass (not Tile): every cross-engine dependency is a standalone wait_ge
on the issuing engine's queue, since this toolchain's static-DMA lowering
rejects DMA instructions with more than one embedded sync wait.
"""

import numpy as np

from concourse import bass, mybir
from concourse.bass_utils import run_bass_kernel_spmd

N_CORES = 8
B, W = 4096, 16384
ROWS = B // N_CORES  # 512 rows per core
P = 128              # SBUF partitions
NT = ROWS // P       # 4 row-tiles per core
NBINS = 16
OUT_COLS = 31        # 1 + 2 + 4 + 8 + 16
NBUF = 2             # double-buffered input tiles

_nc_cache = None


def _build_kernel(repeat: int = 1):
    """Build the per-core Bass program.

    repeat > 1 re-runs the whole per-core workload that many times inside one
    NEFF (reading the same input rows) -- used only for timing, so the
    constant launch/dispatch overhead can be subtracted via the slope.
    """
    nc = bass.Bass()
    x = nc.declare_dram_parameter("x", [ROWS, W], mybir.dt.float32, isOutput=False)
    out = nc.declare_dram_parameter(
        "out", [ROWS, OUT_COLS], mybir.dt.float32, isOutput=True
    )

    NITER = NT * repeat

    with (
        nc.sbuf_tensor("xt0", [P, W], mybir.dt.float32) as xt0,
        nc.sbuf_tensor("xt1", [P, W], mybir.dt.float32) as xt1,
        nc.sbuf_tensor("res", [P, NT * OUT_COLS], mybir.dt.float32) as res,
        nc.semaphore("ld0") as ld0,   # slot-0 load completions (i even)
        nc.semaphore("ld1") as ld1,   # slot-1 load completions (i odd)
        nc.semaphore("st") as st,     # final store completion
        nc.semaphore("vs") as vs,     # DVE reduce progress (5 per row-tile)
        nc.Block() as block,
    ):
        slots = [xt0, xt1]
        ld = [ld0, ld1]

        @block.gpsimd
        def _(gpsimd):
            for i in range(NITER):
                if i >= NBUF:
                    # WAR: slot's previous tile must have been consumed by
                    # its big reduce (reduce 1 of 5) before overwrite.
                    gpsimd.wait_ge(vs, 5 * (i - NBUF) + 1)
                gpsimd.dma_start(
                    slots[i % NBUF][:, :],
                    x[(i % NT) * P : (i % NT + 1) * P, :],
                ).then_inc(ld[i % NBUF], 16)
            gpsimd.wait_ge(vs, 5 * NITER)
            gpsimd.dma_start(
                out[:].rearrange("(n p) c -> p n c", p=P),
                res[:].rearrange("p (n c) -> p n c", n=NT),
            ).then_inc(st, 16)
            gpsimd.wait_ge(st, 16)

        @block.vector
        def _(vector):
            nvs = 0  # running count of vs increments
            for i in range(NITER):
                vector.wait_ge(ld[i % NBUF], 16 * (i // NBUF + 1))
                t = slots[i % NBUF]
                o = res[:, (i % NT) * OUT_COLS : (i % NT + 1) * OUT_COLS]
                # 16 bins of 1024, then pairwise-max down to 8/4/2/1 bins.
                vector.reduce_max(
                    o[:, 15:31],
                    t[:, :].rearrange("p (b w) -> p b w", b=NBINS),
                    axis=mybir.AxisListType.X,
                ).then_inc(vs, 1)
                nvs += 1
                lo, size = 15, 16
                while size > 1:
                    size //= 2
                    # DVE ops don't interlock; wait for the previous level's
                    # writes to land before reading them.
                    vector.wait_ge(vs, nvs)
                    vector.reduce_max(
                        o[:, lo - size : lo],
                        o[:, lo : lo + 2 * size].rearrange(
                            "p (b t) -> p b t", t=2
                        ),
                        axis=mybir.AxisListType.X,
                    ).then_inc(vs, 1)
                    nvs += 1
                    lo -= size

    return nc


def kernel(x: np.ndarray) -> np.ndarray:
    global _nc_cache
    if _nc_cache is None:
        _nc_cache = _build_kernel()
    nc = _nc_cache

    x = np.ascontiguousarray(x, dtype=np.float32)
    in_maps = [{"x": x[c * ROWS : (c + 1) * ROWS]} for c in range(N_CORES)]
    res = run_bass_kernel_spmd(nc, in_maps, core_ids=list(range(N_CORES)))
    return np.concatenate(
        [res.results[c]["out"] for c in range(N_CORES)], axis=0
    )
